# revision 18
# baseline (speedup 1.0000x reference)
"""Trainium2 Bass kernel for a 2-layer GCN forward pass (8 NeuronCores).

    h   = relu(spmm(A, x @ W1) + b1)
    out = softmax(spmm(A, h @ W2) + b2)     spmm(A, h @ W2) == spmm(A, h) @ W2

Distribution (per the sharding hint): nodes are sharded across the 8
cores (graph/data parallel); W1/W2/bias replicated; the all-to-all
gather of source-node features for cross-partition edges is performed
by the host between kernels (it plays the interconnect: pure fp16 row
routing, zero arithmetic).  All arithmetic runs on device:

  K1: support = x @ W1 for the core's own node shard           (PE)
  host: all-to-all -> exp1[slot] = support[src(slot)]          (routing)
  K2: stream exp1 + val-valued one-hot mask slabs, segment-sum
      via mask matmuls (PE, transposed: psumT[64,64] += G.T@S),
      relu+bias on ACT (per-partition bias), then t2 = h @ W2
      fused per tile -> t2 shard                               (PE/ACT)
  host: all-to-all -> exp2[slot] = t2[src(slot)]               (routing)
  K3: stream exp2 + the same mask slabs, segment-sum
      (psum[64,16] += S.T @ G), + b2, softmax -> output shard  (PE/DVE/ACT)

Slot schedule: per (core, dst-tile of 64 nodes) the incident edges are
packed into groups of 128 slots (lane = partition).  One matmul per
group.  The masks (raw edge_val scattered at (lane, group, dst-row),
zeros elsewhere) are static host data streamed from HBM - the same
tensor serves both layers.  Pad slots are all-zero mask columns.
"""
import heapq
import os
import sys
import time

for _p in ("/opt/trn_rl_repo", "/opt/pypackages"):
    if _p not in sys.path:
        sys.path.append(_p)

import numpy as np
from concourse import bacc, mybir, tile, bass_utils

F32 = mybir.dt.float32
F16 = mybir.dt.float16
AX = mybir.AxisListType.X
MUL = mybir.AluOpType.mult
IEQ = mybir.AluOpType.is_equal
ADD = mybir.AluOpType.add
EXP = mybir.ActivationFunctionType.Exp
CPY = mybir.ActivationFunctionType.Copy
RELU = mybir.ActivationFunctionType.Relu

P = 128


class Cfg:
    def __init__(self, n_nodes=100000, f_in=512, hidden=64, n_class=16,
                 n_cores=8, tw=64, ct=8, st1=14):
        self.n_nodes, self.f_in, self.hidden, self.n_class = \
            n_nodes, f_in, hidden, n_class
        self.n_cores, self.tw, self.ct, self.st1 = n_cores, tw, ct, st1
        assert n_nodes % n_cores == 0
        self.npc = n_nodes // n_cores
        self.tpc = -(-self.npc // tw)          # dst tiles (tw rows each)
        self.rows_pad = self.tpc * tw
        self.n_chunks = -(-self.tpc // ct)
        assert f_in % P == 0
        self.kb = f_in // P
        self.tp1 = -(-self.rows_pad // P)      # k1 tiles (128 rows each)
        self.rows1 = self.tp1 * P


class Sched:
    """Slot schedule shared by both spmm layers (identical on all cores
    up to data; group counts are maxed over cores so one program runs
    SPMD on all 8)."""

    def __init__(self, cfg: Cfg, edge_src, edge_dst, edge_val):
        self.cfg = cfg
        ncr, tpc, tw = cfg.n_cores, cfg.tpc, cfg.tw

        core = edge_dst // cfg.npc
        dl = edge_dst % cfg.npc

        # binpack each core's nodes into tiles (LPT with tile 0 biased to
        # absorb the per-core excess) so per-tile group counts are minimal
        # and aligned across cores.  Pure permutation, undone on output.
        t_of = np.zeros((ncr, cfg.npc), np.int32)
        r_of = np.zeros((ncr, cfg.npc), np.int32)
        maxcnt = np.zeros(tpc, np.int64)
        for c in range(ncr):
            deg = np.bincount(dl[core == c], minlength=cfg.npc)
            order_n = np.argsort(-deg, kind="stable")
            bias = np.zeros(tpc, np.int64)
            bias[0] = -(tw * P // 16)
            heap = [(int(bias[t]), 0, t) for t in range(tpc)]
            heapq.heapify(heap)
            rows_t = np.zeros(tpc, np.int32)
            load = np.zeros(tpc, np.int64)
            for n in order_n:
                while True:
                    _, _, t = heapq.heappop(heap)
                    if rows_t[t] < tw:
                        break
                t_of[c, n] = t
                r_of[c, n] = rows_t[t]
                rows_t[t] += 1
                load[t] += deg[n]
                if rows_t[t] < tw:
                    heapq.heappush(heap, (int(load[t] + bias[t]),
                                          int(rows_t[t]), t))
            maxcnt = np.maximum(maxcnt, load)
        self.outrow = t_of.astype(np.int64) * tw + r_of

        tile_e = t_of[core, dl]
        row_e = r_of[core, dl]

        g_t = np.maximum(-(-maxcnt // P), 1)          # groups per tile
        gbase = np.concatenate([[0], np.cumsum(g_t)])
        self.GT = int(gbase[-1])

        order = np.lexsort((edge_src, tile_e, core))
        core_s, tile_s = core[order], tile_e[order]
        src_s, row_s, val_s = edge_src[order], row_e[order], edge_val[order]

        key = core_s * tpc + tile_s
        E = len(key)
        change = np.r_[True, key[1:] != key[:-1]]
        starts = np.flatnonzero(change)
        sizes = np.diff(np.r_[starts, E])
        rank = np.arange(E) - np.repeat(starts, sizes)

        slot = gbase[tile_s] * P + rank               # within-core slot id
        lane = slot % P
        grp = slot // P

        # val-valued one-hot mask slabs: sval[c, lane, grp, dstrow] = edge_val
        # (raw input values placed into the slot layout; zeros elsewhere)
        self.sval = np.zeros((ncr, P, self.GT, tw), np.float16)
        self.sval[core_s, lane, grp, row_s] = val_s.astype(np.float16)
        self.dst_w = np.full((ncr, P, self.GT), 255.0, np.float16)
        self.dst_w[core_s, lane, grp] = row_s.astype(np.float16)
        self.val_w = np.zeros((ncr, P, self.GT), np.float16)
        self.val_w[core_s, lane, grp] = val_s.astype(np.float16)
        self.srcslot = np.zeros((ncr, P, self.GT), np.int32)
        self.srcslot[core_s, lane, grp] = src_s

        # chunks of ct tiles
        self.chunks = []
        for i in range(cfg.n_chunks):
            t0 = i * cfg.ct
            tiles = list(range(t0, min(t0 + cfg.ct, tpc)))
            goff = int(gbase[t0])
            ops = [(int(gbase[t] - goff), int(gbase[t + 1] - goff))
                   for t in tiles]
            Gc = int(gbase[tiles[-1] + 1] - goff)
            self.chunks.append(dict(tiles=tiles, goff=goff, Gc=Gc, ops=ops,
                                    idx=i))
        self.Gc_max = max(ch["Gc"] for ch in self.chunks)


# ---------------------------------------------------------------- kernels
def build_k1(cfg: Cfg):
    """support = x @ W1, node-sharded.  xt is host-pre-transposed:
    xt[pj, t, kb*128+pi] = x[t*128+pi, kb*128+pj].  x is cast f32->f16
    during the (SWDGE) DMA; matmuls run fp16."""
    H = cfg.hidden
    nc = bacc.Bacc(None, target_bir_lowering=False)
    xt_d = nc.dram_tensor("xt", [P, cfg.tp1, cfg.f_in], F32,
                          kind="ExternalInput")
    w1_d = nc.dram_tensor("w1", [cfg.f_in, H], F16, kind="ExternalInput")
    sup_d = nc.dram_tensor("sup", [P, cfg.tp1 * H], F16,
                           kind="ExternalOutput")

    ST = cfg.st1
    with tile.TileContext(nc) as tc:
        with (
            tc.tile_pool(name="const", bufs=1) as cpool,
            tc.tile_pool(name="xload", bufs=4) as xpool,
            tc.tile_pool(name="sout", bufs=2) as opool,
            tc.tile_pool(name="ps", bufs=8, space="PSUM") as pspool,
        ):
            w1_t = cpool.tile([P, cfg.kb, H], F16)
            nc.sync.dma_start(w1_t[:],
                              w1_d[:].rearrange("(kb p) n -> p kb n", p=P))
            blocks = [(0, min(4, cfg.tp1))]
            while blocks[-1][0] + blocks[-1][1] < cfg.tp1:
                b0 = blocks[-1][0] + blocks[-1][1]
                blocks.append((b0, min(ST, cfg.tp1 - b0)))
            for t0, n_t in blocks:
                xsb = xpool.tile([P, n_t, cfg.f_in], F16, tag="xsb")
                nc.gpsimd.dma_start(xsb[:], xt_d[:, t0:t0 + n_t, :])
                osb = opool.tile([P, n_t, H], F16, tag="osb")
                for tl in range(n_t):
                    ps = pspool.tile([P, H], F32, tag="ps1")
                    for kb in range(cfg.kb):
                        nc.tensor.matmul(
                            ps[:], xsb[:, tl, kb * P:(kb + 1) * P],
                            w1_t[:, kb, :], start=(kb == 0),
                            stop=(kb == cfg.kb - 1))
                    nc.scalar.activation(osb[:, tl, :], ps[:], CPY)
                nc.sync.dma_start(
                    sup_d[:, t0 * H:(t0 + n_t) * H],
                    osb[:].rearrange("p t n -> p (t n)"))
    nc.compile()
    return nc


def build_k2(cfg: Cfg, sch: Sched):
    """Layer 1 spmm + relu + bias, fused with t2 = h @ W2.

    Streams exp1 (host-routed fp16 slot rows) and the val-valued mask
    slabs.  Transposed segment-sum:
    psumT[64 feat, 64 dst] += G[128 slot, 64 feat].T @ S[128 slot, 64 dst].
    Epilogue per tile: ACT relu(psumT + b1) -> hT (f32), then
    psB[16, tl*64:..] = W2.T @ hT.  Output t2T [16, tpc*64] f32."""
    H, C, tw, ct = cfg.hidden, cfg.n_class, cfg.tw, cfg.ct
    nc = bacc.Bacc(None, target_bir_lowering=False)
    exp_d = nc.dram_tensor("exp1", [P, sch.GT * H], F16, kind="ExternalInput")
    sv_d = nc.dram_tensor("sval", [P, sch.GT * tw], F16, kind="ExternalInput")
    dst_d = nc.dram_tensor("dstw", [P, sch.GT], F16, kind="ExternalInput")
    val_d = nc.dram_tensor("valw", [P, sch.GT], F16, kind="ExternalInput")
    iota_d = nc.dram_tensor("iota", [P, tw], F16, kind="ExternalInput")
    b1_d = nc.dram_tensor("b1c", [H, 1], F32, kind="ExternalInput")
    w2_d = nc.dram_tensor("w2c", [H, C], F32, kind="ExternalInput")
    out_d = nc.dram_tensor("t2T", [C, cfg.tpc * tw], F32,
                           kind="ExternalOutput")
    TB = 4  # tiles per psum group (ACT batching)

    with tile.TileContext(nc) as tc:
        with (
            tc.tile_pool(name="const", bufs=1) as cpool,
            tc.tile_pool(name="gath", bufs=4) as gpool,
            tc.tile_pool(name="seg", bufs=4) as spool,
            tc.tile_pool(name="ht", bufs=2) as hpool,
            tc.tile_pool(name="ot", bufs=2) as opool,
            tc.tile_pool(name="psA", bufs=6, space="PSUM") as psA,
            tc.tile_pool(name="psB", bufs=2, space="PSUM") as psB,
        ):
            b1_t = cpool.tile([H, 1], F32)
            w2_t = cpool.tile([H, C], F32)
            dst_t = cpool.tile([P, sch.GT], F16)
            val_t = cpool.tile([P, sch.GT], F16)
            iota_t = cpool.tile([P, tw], F16)
            nc.gpsimd.dma_start(b1_t[:], b1_d[:])
            nc.gpsimd.dma_start(w2_t[:], w2_d[:])
            nc.gpsimd.dma_start(dst_t[:], dst_d[:])
            nc.gpsimd.dma_start(val_t[:], val_d[:])
            nc.gpsimd.dma_start(iota_t[:], iota_d[:])

            for ch in sch.chunks:
                tiles, goff, Gc = ch["tiles"], ch["goff"], ch["Gc"]
                n_t = len(tiles)
                gt = gpool.tile([P, sch.Gc_max, H], F16, tag="gt")
                nc.sync.dma_start(
                    gt[:, :Gc, :].rearrange("p g n -> p (g n)"),
                    exp_d[:, goff * H:(goff + Gc) * H])
                st = spool.tile([P, sch.Gc_max, tw], F16, tag="st")
                if ch["idx"] % 2 == 1:
                    nc.vector.tensor_tensor(
                        st[:, :Gc, :],
                        dst_t[:, goff:goff + Gc].unsqueeze(2)
                            .broadcast_to([P, Gc, tw]),
                        iota_t[:].unsqueeze(1).broadcast_to([P, Gc, tw]),
                        op=IEQ)
                    nc.vector.tensor_tensor(
                        st[:, :Gc, :], st[:, :Gc, :],
                        val_t[:, goff:goff + Gc].unsqueeze(2)
                            .broadcast_to([P, Gc, tw]), op=MUL)
                else:
                    nc.sync.dma_start(
                        st[:, :Gc, :].rearrange("p g n -> p (g n)"),
                        sv_d[:, goff * tw:(goff + Gc) * tw])

                hT = hpool.tile([H, ct, tw], F32, tag="hT")
                ps2 = psB.tile([C, ct * tw], F32, tag="t2")
                for q0 in range(0, n_t, TB):
                    qn = min(TB, n_t - q0)
                    ps = psA.tile([H, TB * tw], F32, tag="agg")
                    for tl in range(q0, q0 + qn):
                        lo, hi = ch["ops"][tl]
                        sl = ps[:, (tl - q0) * tw:(tl - q0 + 1) * tw]
                        if hi == lo:
                            nc.vector.memset(sl, 0.0)
                        for k in range(lo, hi):
                            nc.tensor.matmul(sl, gt[:, k, :], st[:, k, :],
                                             start=(k == lo),
                                             stop=(k == hi - 1))
                    nc.scalar.activation(
                        hT[:, q0:q0 + qn, :].rearrange("h t w -> h (t w)"),
                        ps[:, :qn * tw], RELU, bias=b1_t[:])
                nc.tensor.matmul(
                    ps2[:, :n_t * tw],
                    w2_t[:],
                    hT[:, :n_t, :].rearrange("h t w -> h (t w)"),
                    start=True, stop=True)
                oT = opool.tile([C, ct * tw], F32, tag="oT")
                nc.scalar.activation(oT[:, :n_t * tw], ps2[:, :n_t * tw], CPY)
                t0 = tiles[0]
                nc.scalar.dma_start(
                    out_d[:, t0 * tw:(t0 + n_t) * tw], oT[:, :n_t * tw])
    nc.compile()
    return nc


def build_k3(cfg: Cfg, sch: Sched):
    """Layer 2 spmm + b2 + softmax.  Streams exp2 (fp16 slot rows of
    t2 = h @ W2) and the same mask slabs.
    psum[64 dst, 16] += S[128 slot, 64 dst].T @ G[128, 16],
    packed per chunk into psC[64, ct*16]."""
    C, tw, ct = cfg.n_class, cfg.tw, cfg.ct
    nc = bacc.Bacc(None, target_bir_lowering=False)
    exp_d = nc.dram_tensor("exp2", [P, sch.GT * C], F16, kind="ExternalInput")
    sv_d = nc.dram_tensor("sval", [P, sch.GT * tw], F16, kind="ExternalInput")
    b2_d = nc.dram_tensor("b2r", [tw, ct * C], F32, kind="ExternalInput")
    out_d = nc.dram_tensor("oout", [tw, cfg.tpc * C], F32,
                           kind="ExternalOutput")

    with tile.TileContext(nc) as tc:
        with (
            tc.tile_pool(name="const", bufs=1) as cpool,
            tc.tile_pool(name="gath", bufs=6) as gpool,
            tc.tile_pool(name="seg", bufs=6) as spool,
            tc.tile_pool(name="epi", bufs=3) as epool,
            tc.tile_pool(name="psC", bufs=4, space="PSUM") as psC,
        ):
            b2_t = cpool.tile([tw, ct * C], F32)
            nc.gpsimd.dma_start(b2_t[:], b2_d[:])

            for ch in sch.chunks:
                tiles, goff, Gc = ch["tiles"], ch["goff"], ch["Gc"]
                n_t = len(tiles)
                gt = gpool.tile([P, sch.Gc_max, C], F16, tag="gt")
                nc.sync.dma_start(
                    gt[:, :Gc, :].rearrange("p g n -> p (g n)"),
                    exp_d[:, goff * C:(goff + Gc) * C])
                st = spool.tile([P, sch.Gc_max, tw], F16, tag="st")
                nc.sync.dma_start(
                    st[:, :Gc, :].rearrange("p g n -> p (g n)"),
                    sv_d[:, goff * tw:(goff + Gc) * tw])

                ps = psC.tile([tw, ct * C], F32, tag="lg")
                for tl in range(n_t):
                    lo, hi = ch["ops"][tl]
                    if hi == lo:
                        nc.vector.memset(ps[:, tl * C:(tl + 1) * C], 0.0)
                    for k in range(lo, hi):
                        nc.tensor.matmul(ps[:, tl * C:(tl + 1) * C],
                                         st[:, k, :], gt[:, k, :],
                                         start=(k == lo), stop=(k == hi - 1))
                hsb = epool.tile([tw, ct, C], F32, tag="hsb")
                flat = hsb[:].rearrange("p t n -> p (t n)")
                nc.vector.tensor_tensor(flat[:, :n_t * C], ps[:, :n_t * C],
                                        b2_t[:, :n_t * C], op=ADD)
                nm = epool.tile([tw, ct], F32, tag="nm")
                nc.vector.reduce_max(nm[:, :n_t], hsb[:, :n_t, :], axis=AX,
                                     negate=True)
                nc.vector.tensor_tensor(
                    hsb[:, :n_t, :], hsb[:, :n_t, :],
                    nm[:, :n_t].unsqueeze(2).broadcast_to([tw, n_t, C]),
                    op=ADD)
                nc.scalar.activation(flat[:, :n_t * C], flat[:, :n_t * C], EXP)
                se = epool.tile([tw, ct], F32, tag="se")
                nc.vector.reduce_sum(se[:, :n_t], hsb[:, :n_t, :], axis=AX)
                ri = epool.tile([tw, ct], F32, tag="ri")
                nc.vector.reciprocal(ri[:, :n_t], se[:, :n_t])
                nc.vector.tensor_tensor(
                    hsb[:, :n_t, :], hsb[:, :n_t, :],
                    ri[:, :n_t].unsqueeze(2).broadcast_to([tw, n_t, C]),
                    op=MUL)
                t0 = tiles[0]
                nc.scalar.dma_start(out_d[:, t0 * C:(t0 + n_t) * C],
                                    flat[:, :n_t * C])
    nc.compile()
    return nc


# ---------------------------------------------------------------- driver
LAST_PROFILE = {}


def _run(nc, in_maps, label):
    trace = os.environ.get("GCN_PROFILE") == "1"
    t0 = time.time()
    for attempt in range(3):
        try:
            res = bass_utils.run_bass_kernel_spmd(
                nc, in_maps, core_ids=list(range(len(in_maps))), trace=trace)
            break
        except Exception:
            if attempt == 2:
                raise
            time.sleep(5)
    LAST_PROFILE[label] = dict(
        wall_s=time.time() - t0,
        exec_time_ns=res.exec_time_ns,
        trace=(res.instructions_and_trace or (None, None))[1])
    return res.results


def gcn_forward(cfg: Cfg, x, edge_src, edge_dst, edge_val, W1, b1, W2, b2):
    ncr, H, C, tw, ct = cfg.n_cores, cfg.hidden, cfg.n_class, cfg.tw, cfg.ct
    x = np.asarray(x, np.float32)
    W1 = np.asarray(W1, np.float32)
    b1 = np.asarray(b1, np.float32)
    W2 = np.asarray(W2, np.float32)
    b2 = np.asarray(b2, np.float32)
    edge_src = np.asarray(edge_src, np.int64)
    edge_dst = np.asarray(edge_dst, np.int64)
    edge_val = np.asarray(edge_val, np.float32)

    t0 = time.time()
    sch = Sched(cfg, edge_src, edge_dst, edge_val)
    b1c = b1.reshape(H, 1)
    b2r = np.tile(b2, (tw, ct))
    sval = sch.sval.reshape(ncr, P, sch.GT * tw)
    iota = np.tile(np.arange(tw, dtype=np.float16), (P, 1))
    LAST_PROFILE["prep_s"] = time.time() - t0
    LAST_PROFILE["sched"] = dict(GT=sch.GT, Gc_max=sch.Gc_max,
                                 slots=sch.GT * P,
                                 n_edges=len(edge_src) // ncr)

    # K1: support = x @ W1 (own shard)
    in1 = []
    for c in range(ncr):
        xs = x[c * cfg.npc:(c + 1) * cfg.npc]
        xp = np.zeros((cfg.rows1, cfg.f_in), np.float32)
        xp[:cfg.npc] = xs
        xt = np.ascontiguousarray(
            xp.reshape(cfg.tp1, P, cfg.kb, P).transpose(3, 0, 2, 1)
              .reshape(P, cfg.tp1, cfg.f_in))
        in1.append(dict(xt=xt, w1=W1.astype(np.float16)))
    nc1 = build_k1(cfg)
    r1 = _run(nc1, in1, "k1")

    # host all-to-all #1: route support rows into slot order (fp16, no math)
    sup = np.concatenate(
        [r1[c]["sup"].reshape(P, cfg.tp1, H).transpose(1, 0, 2)
         .reshape(cfg.rows1, H)[:cfg.npc] for c in range(ncr)], axis=0)
    in2 = [dict(exp1=np.ascontiguousarray(
                    sup[sch.srcslot[c]].reshape(P, sch.GT * H)),
                sval=sval[c], dstw=sch.dst_w[c], valw=sch.val_w[c],
                iota=iota, b1c=b1c, w2c=W2)
           for c in range(ncr)]
    nc2 = build_k2(cfg, sch)
    r2 = _run(nc2, in2, "k2")

    # host all-to-all #2: route t2 rows into slot order
    t2 = np.concatenate(
        [r2[c]["t2T"].T[sch.outrow[c]] for c in range(ncr)],
        axis=0).astype(np.float16)
    in3 = [dict(exp2=np.ascontiguousarray(
                    t2[sch.srcslot[c]].reshape(P, sch.GT * C)),
                sval=sval[c], b2r=b2r)
           for c in range(ncr)]
    nc3 = build_k3(cfg, sch)
    r3 = _run(nc3, in3, "k3")

    out = np.concatenate(
        [r3[c]["oout"].reshape(tw, cfg.tpc, C).transpose(1, 0, 2)
         .reshape(cfg.rows_pad, C)[sch.outrow[c]] for c in range(ncr)],
        axis=0)
    return out


def kernel(x, edge_src, edge_dst, edge_val, W1, b1, W2, b2):
    cfg = Cfg()
    return gcn_forward(cfg, x, edge_src, edge_dst, edge_val, W1, b1, W2, b2)


# ---------------------------------------------------------------- self test
def _numpy_ref(x, es, ed, ev, W1, b1, W2, b2, n):
    def spmm(d):
        g = d[es] * ev[:, None]
        out = np.zeros((n, d.shape[1]), np.float32)
        np.add.at(out, ed, g)
        return out
    h = spmm(x @ W1) + b1
    h = np.maximum(h, 0)
    lg = spmm(h) @ W2 + b2
    e = np.exp(lg - lg.max(1, keepdims=True))
    return e / e.sum(1, keepdims=True)


def _selftest():
    cfg = Cfg(n_nodes=4096, f_in=256, hidden=64, n_class=16,
              n_cores=8, tw=64, ct=4, st1=4)
    rng = np.random.default_rng(1)
    n_edges = 65536
    x = rng.standard_normal((cfg.n_nodes, cfg.f_in), dtype=np.float32)
    es = rng.integers(0, cfg.n_nodes, n_edges)
    ed = rng.integers(0, cfg.n_nodes, n_edges)
    ev = rng.random(n_edges, dtype=np.float32)
    W1 = rng.standard_normal((cfg.f_in, cfg.hidden), dtype=np.float32) * 0.125
    b1 = rng.standard_normal(cfg.hidden, dtype=np.float32) * 0.01
    W2 = rng.standard_normal((cfg.hidden, cfg.n_class), dtype=np.float32) * 0.25
    b2 = rng.standard_normal(cfg.n_class, dtype=np.float32) * 0.01
    act = gcn_forward(cfg, x, es, ed, ev, W1, b1, W2, b2)
    ref = _numpy_ref(x, es, ed, ev, W1, b1, W2, b2, cfg.n_nodes)
    err = np.abs(act - ref).max()
    rel = err / np.abs(ref).max()
    print(f"selftest absmax={err:.3e} relmax={rel:.3e}")
    print("profile:", LAST_PROFILE)
    assert rel < 1.2e-2, "SELFTEST FAIL"
    print("SELFTEST PASS")


if __name__ == "__main__":
    _selftest()


# revision 19
# speedup vs baseline: 1.0776x; 1.0776x over previous
"""Trainium2 Bass kernel for a 2-layer GCN forward pass (8 NeuronCores).

    h   = relu(spmm(A, x @ W1) + b1)
    out = softmax(spmm(A, h @ W2) + b2)     spmm(A, h @ W2) == spmm(A, h) @ W2

Distribution (per the sharding hint): nodes are sharded across the 8
cores (graph/data parallel); W1/W2/bias replicated; the all-to-all
gather of source-node features for cross-partition edges is performed
by the host between kernels (it plays the interconnect: pure fp16 row
routing, zero arithmetic).  All arithmetic runs on device:

  K1: support = x @ W1 for the core's own node shard           (PE)
  host: all-to-all -> exp1[slot] = support[src(slot)]          (routing)
  K2: stream exp1 + val-valued one-hot mask slabs, segment-sum
      via mask matmuls (PE, transposed: psumT[64,64] += G.T@S),
      relu+bias on ACT (per-partition bias), then t2 = h @ W2
      fused per tile -> t2 shard                               (PE/ACT)
  host: all-to-all -> exp2[slot] = t2[src(slot)]               (routing)
  K3: stream exp2 + the same mask slabs, segment-sum
      (psum[64,16] += S.T @ G), + b2, softmax -> output shard  (PE/DVE/ACT)

Slot schedule: per (core, dst-tile of 64 nodes) the incident edges are
packed into groups of 128 slots (lane = partition).  One matmul per
group.  The masks (raw edge_val scattered at (lane, group, dst-row),
zeros elsewhere) are static host data streamed from HBM - the same
tensor serves both layers.  Pad slots are all-zero mask columns.
"""
import heapq
import os
import sys
import time

for _p in ("/opt/trn_rl_repo", "/opt/pypackages"):
    if _p not in sys.path:
        sys.path.append(_p)

import numpy as np
from concourse import bacc, mybir, tile, bass_utils

F32 = mybir.dt.float32
F16 = mybir.dt.float16
AX = mybir.AxisListType.X
MUL = mybir.AluOpType.mult
IEQ = mybir.AluOpType.is_equal
ADD = mybir.AluOpType.add
EXP = mybir.ActivationFunctionType.Exp
CPY = mybir.ActivationFunctionType.Copy
RELU = mybir.ActivationFunctionType.Relu

P = 128


class Cfg:
    def __init__(self, n_nodes=100000, f_in=512, hidden=64, n_class=16,
                 n_cores=8, tw=64, ct=8, st1=14):
        self.n_nodes, self.f_in, self.hidden, self.n_class = \
            n_nodes, f_in, hidden, n_class
        self.n_cores, self.tw, self.ct, self.st1 = n_cores, tw, ct, st1
        assert n_nodes % n_cores == 0
        self.npc = n_nodes // n_cores
        self.tpc = -(-self.npc // tw)          # dst tiles (tw rows each)
        self.rows_pad = self.tpc * tw
        self.n_chunks = -(-self.tpc // ct)
        assert f_in % P == 0
        self.kb = f_in // P
        self.tp1 = -(-self.rows_pad // P)      # k1 tiles (128 rows each)
        self.rows1 = self.tp1 * P


class Sched:
    """Slot schedule shared by both spmm layers (identical on all cores
    up to data; group counts are maxed over cores so one program runs
    SPMD on all 8)."""

    def __init__(self, cfg: Cfg, edge_src, edge_dst, edge_val):
        self.cfg = cfg
        ncr, tpc, tw = cfg.n_cores, cfg.tpc, cfg.tw

        core = edge_dst // cfg.npc
        dl = edge_dst % cfg.npc

        # binpack each core's nodes into tiles (LPT with tile 0 biased to
        # absorb the per-core excess) so per-tile group counts are minimal
        # and aligned across cores.  Pure permutation, undone on output.
        t_of = np.zeros((ncr, cfg.npc), np.int32)
        r_of = np.zeros((ncr, cfg.npc), np.int32)
        maxcnt = np.zeros(tpc, np.int64)
        for c in range(ncr):
            deg = np.bincount(dl[core == c], minlength=cfg.npc)
            order_n = np.argsort(-deg, kind="stable")
            bias = np.zeros(tpc, np.int64)
            bias[0] = -(tw * P // 16)
            heap = [(int(bias[t]), 0, t) for t in range(tpc)]
            heapq.heapify(heap)
            rows_t = np.zeros(tpc, np.int32)
            load = np.zeros(tpc, np.int64)
            for n in order_n:
                while True:
                    _, _, t = heapq.heappop(heap)
                    if rows_t[t] < tw:
                        break
                t_of[c, n] = t
                r_of[c, n] = rows_t[t]
                rows_t[t] += 1
                load[t] += deg[n]
                if rows_t[t] < tw:
                    heapq.heappush(heap, (int(load[t] + bias[t]),
                                          int(rows_t[t]), t))
            maxcnt = np.maximum(maxcnt, load)
        self.outrow = t_of.astype(np.int64) * tw + r_of

        tile_e = t_of[core, dl]
        row_e = r_of[core, dl]

        g_t = np.maximum(-(-maxcnt // P), 1)          # groups per tile
        gbase = np.concatenate([[0], np.cumsum(g_t)])
        self.GT = int(gbase[-1])

        order = np.lexsort((edge_src, tile_e, core))
        core_s, tile_s = core[order], tile_e[order]
        src_s, row_s, val_s = edge_src[order], row_e[order], edge_val[order]

        key = core_s * tpc + tile_s
        E = len(key)
        change = np.r_[True, key[1:] != key[:-1]]
        starts = np.flatnonzero(change)
        sizes = np.diff(np.r_[starts, E])
        rank = np.arange(E) - np.repeat(starts, sizes)

        slot = gbase[tile_s] * P + rank               # within-core slot id
        lane = slot % P
        grp = slot // P

        # val-valued one-hot mask slabs: sval[c, lane, grp, dstrow] = edge_val
        # (raw input values placed into the slot layout; zeros elsewhere)
        self.sval = np.zeros((ncr, P, self.GT, tw), np.float16)
        self.sval[core_s, lane, grp, row_s] = val_s.astype(np.float16)
        self.dst_w = np.full((ncr, P, self.GT), 255.0, np.float16)
        self.dst_w[core_s, lane, grp] = row_s.astype(np.float16)
        self.val_w = np.zeros((ncr, P, self.GT), np.float16)
        self.val_w[core_s, lane, grp] = val_s.astype(np.float16)
        self.srcslot = np.zeros((ncr, P, self.GT), np.int32)
        self.srcslot[core_s, lane, grp] = src_s

        # chunks of ct tiles
        self.chunks = []
        for i in range(cfg.n_chunks):
            t0 = i * cfg.ct
            tiles = list(range(t0, min(t0 + cfg.ct, tpc)))
            goff = int(gbase[t0])
            ops = [(int(gbase[t] - goff), int(gbase[t + 1] - goff))
                   for t in tiles]
            Gc = int(gbase[tiles[-1] + 1] - goff)
            self.chunks.append(dict(tiles=tiles, goff=goff, Gc=Gc, ops=ops,
                                    idx=i))
        self.Gc_max = max(ch["Gc"] for ch in self.chunks)


# ---------------------------------------------------------------- kernels
def build_k1(cfg: Cfg):
    """support = x @ W1, node-sharded.  xt is host-pre-transposed:
    xt[pj, t, kb*128+pi] = x[t*128+pi, kb*128+pj].  x is cast f32->f16
    during the (SWDGE) DMA; matmuls run fp16."""
    H = cfg.hidden
    nc = bacc.Bacc(None, target_bir_lowering=False)
    xt_d = nc.dram_tensor("xt", [P, cfg.tp1, cfg.f_in], F32,
                          kind="ExternalInput")
    w1_d = nc.dram_tensor("w1", [cfg.f_in, H], F16, kind="ExternalInput")
    sup_d = nc.dram_tensor("sup", [P, cfg.tp1 * H], F16,
                           kind="ExternalOutput")

    ST = cfg.st1
    with tile.TileContext(nc) as tc:
        with (
            tc.tile_pool(name="const", bufs=1) as cpool,
            tc.tile_pool(name="xload", bufs=4) as xpool,
            tc.tile_pool(name="sout", bufs=2) as opool,
            tc.tile_pool(name="ps", bufs=8, space="PSUM") as pspool,
        ):
            w1_t = cpool.tile([P, cfg.kb, H], F16)
            nc.sync.dma_start(w1_t[:],
                              w1_d[:].rearrange("(kb p) n -> p kb n", p=P))
            blocks = [(0, min(4, cfg.tp1))]
            while blocks[-1][0] + blocks[-1][1] < cfg.tp1:
                b0 = blocks[-1][0] + blocks[-1][1]
                blocks.append((b0, min(ST, cfg.tp1 - b0)))
            for t0, n_t in blocks:
                xsb = xpool.tile([P, n_t, cfg.f_in], F16, tag="xsb")
                nc.gpsimd.dma_start(xsb[:], xt_d[:, t0:t0 + n_t, :])
                osb = opool.tile([P, n_t, H], F16, tag="osb")
                for tl in range(n_t):
                    ps = pspool.tile([P, H], F32, tag="ps1")
                    for kb in range(cfg.kb):
                        nc.tensor.matmul(
                            ps[:], xsb[:, tl, kb * P:(kb + 1) * P],
                            w1_t[:, kb, :], start=(kb == 0),
                            stop=(kb == cfg.kb - 1))
                    nc.scalar.activation(osb[:, tl, :], ps[:], CPY)
                nc.sync.dma_start(
                    sup_d[:, t0 * H:(t0 + n_t) * H],
                    osb[:].rearrange("p t n -> p (t n)"))
    nc.compile()
    return nc


def build_k2(cfg: Cfg, sch: Sched):
    """Layer 1 spmm + relu + bias, fused with t2 = h @ W2.

    Streams exp1 (host-routed fp16 slot rows) and the val-valued mask
    slabs.  Transposed segment-sum:
    psumT[64 feat, 64 dst] += G[128 slot, 64 feat].T @ S[128 slot, 64 dst].
    Epilogue per tile: ACT relu(psumT + b1) -> hT (f32), then
    psB[16, tl*64:..] = W2.T @ hT.  Output t2T [16, tpc*64] f32."""
    H, C, tw, ct = cfg.hidden, cfg.n_class, cfg.tw, cfg.ct
    nc = bacc.Bacc(None, target_bir_lowering=False)
    exp_d = nc.dram_tensor("exp1", [P, sch.GT * H], F16, kind="ExternalInput")
    sv_d = nc.dram_tensor("sval", [P, sch.GT * tw], F16, kind="ExternalInput")
    dst_d = nc.dram_tensor("dstw", [P, sch.GT], F16, kind="ExternalInput")
    val_d = nc.dram_tensor("valw", [P, sch.GT], F16, kind="ExternalInput")
    iota_d = nc.dram_tensor("iota", [P, tw], F16, kind="ExternalInput")
    b1_d = nc.dram_tensor("b1c", [H, 1], F32, kind="ExternalInput")
    w2_d = nc.dram_tensor("w2c", [H, C], F32, kind="ExternalInput")
    out_d = nc.dram_tensor("t2T", [C, cfg.tpc * tw], F32,
                           kind="ExternalOutput")
    TB = 4  # tiles per psum group (ACT batching)

    with tile.TileContext(nc) as tc:
        with (
            tc.tile_pool(name="const", bufs=1) as cpool,
            tc.tile_pool(name="gath", bufs=4) as gpool,
            tc.tile_pool(name="seg", bufs=4) as spool,
            tc.tile_pool(name="ht", bufs=2) as hpool,
            tc.tile_pool(name="ot", bufs=2) as opool,
            tc.tile_pool(name="psA", bufs=6, space="PSUM") as psA,
            tc.tile_pool(name="psB", bufs=2, space="PSUM") as psB,
        ):
            b1_t = cpool.tile([H, 1], F32)
            w2_t = cpool.tile([H, C], F32)
            dst_t = cpool.tile([P, sch.GT], F16)
            val_t = cpool.tile([P, sch.GT], F16)
            iota_t = cpool.tile([P, tw], F16)
            nc.sync.dma_start(b1_t[:], b1_d[:])
            nc.sync.dma_start(w2_t[:], w2_d[:])
            nc.sync.dma_start(dst_t[:], dst_d[:])
            nc.sync.dma_start(val_t[:], val_d[:])
            nc.sync.dma_start(iota_t[:], iota_d[:])

            for ch in sch.chunks:
                tiles, goff, Gc = ch["tiles"], ch["goff"], ch["Gc"]
                n_t = len(tiles)
                gt = gpool.tile([P, sch.Gc_max, H], F16, tag="gt")
                nc.sync.dma_start(
                    gt[:, :Gc, :].rearrange("p g n -> p (g n)"),
                    exp_d[:, goff * H:(goff + Gc) * H])
                st = spool.tile([P, sch.Gc_max, tw], F16, tag="st")
                if ch["idx"] % 2 == 1:
                    nc.vector.tensor_tensor(
                        st[:, :Gc, :],
                        dst_t[:, goff:goff + Gc].unsqueeze(2)
                            .broadcast_to([P, Gc, tw]),
                        iota_t[:].unsqueeze(1).broadcast_to([P, Gc, tw]),
                        op=IEQ)
                    nc.vector.tensor_tensor(
                        st[:, :Gc, :], st[:, :Gc, :],
                        val_t[:, goff:goff + Gc].unsqueeze(2)
                            .broadcast_to([P, Gc, tw]), op=MUL)
                else:
                    nc.sync.dma_start(
                        st[:, :Gc, :].rearrange("p g n -> p (g n)"),
                        sv_d[:, goff * tw:(goff + Gc) * tw])

                hT = hpool.tile([H, ct, tw], F32, tag="hT")
                ps2 = psB.tile([C, ct * tw], F32, tag="t2")
                for q0 in range(0, n_t, TB):
                    qn = min(TB, n_t - q0)
                    ps = psA.tile([H, TB * tw], F32, tag="agg")
                    for tl in range(q0, q0 + qn):
                        lo, hi = ch["ops"][tl]
                        sl = ps[:, (tl - q0) * tw:(tl - q0 + 1) * tw]
                        if hi == lo:
                            nc.vector.memset(sl, 0.0)
                        for k in range(lo, hi):
                            nc.tensor.matmul(sl, gt[:, k, :], st[:, k, :],
                                             start=(k == lo),
                                             stop=(k == hi - 1))
                    nc.scalar.activation(
                        hT[:, q0:q0 + qn, :].rearrange("h t w -> h (t w)"),
                        ps[:, :qn * tw], RELU, bias=b1_t[:])
                nc.tensor.matmul(
                    ps2[:, :n_t * tw],
                    w2_t[:],
                    hT[:, :n_t, :].rearrange("h t w -> h (t w)"),
                    start=True, stop=True)
                oT = opool.tile([C, ct * tw], F32, tag="oT")
                nc.scalar.activation(oT[:, :n_t * tw], ps2[:, :n_t * tw], CPY)
                t0 = tiles[0]
                nc.scalar.dma_start(
                    out_d[:, t0 * tw:(t0 + n_t) * tw], oT[:, :n_t * tw])
    nc.compile()
    return nc


def build_k3(cfg: Cfg, sch: Sched):
    """Layer 2 spmm + b2 + softmax.  Streams exp2 (fp16 slot rows of
    t2 = h @ W2) and the same mask slabs.
    psum[64 dst, 16] += S[128 slot, 64 dst].T @ G[128, 16],
    packed per chunk into psC[64, ct*16]."""
    C, tw, ct = cfg.n_class, cfg.tw, cfg.ct
    nc = bacc.Bacc(None, target_bir_lowering=False)
    exp_d = nc.dram_tensor("exp2", [P, sch.GT * C], F16, kind="ExternalInput")
    sv_d = nc.dram_tensor("sval", [P, sch.GT * tw], F16, kind="ExternalInput")
    dst_d = nc.dram_tensor("dstw", [P, sch.GT], F16, kind="ExternalInput")
    val_d = nc.dram_tensor("valw", [P, sch.GT], F16, kind="ExternalInput")
    iota_d = nc.dram_tensor("iota", [P, tw], F16, kind="ExternalInput")
    b2_d = nc.dram_tensor("b2r", [tw, ct * C], F32, kind="ExternalInput")
    out_d = nc.dram_tensor("oout", [tw, cfg.tpc * C], F32,
                           kind="ExternalOutput")

    with tile.TileContext(nc) as tc:
        with (
            tc.tile_pool(name="const", bufs=1) as cpool,
            tc.tile_pool(name="gath", bufs=6) as gpool,
            tc.tile_pool(name="seg", bufs=6) as spool,
            tc.tile_pool(name="epi", bufs=3) as epool,
            tc.tile_pool(name="psC", bufs=4, space="PSUM") as psC,
        ):
            b2_t = cpool.tile([tw, ct * C], F32)
            dst_t = cpool.tile([P, sch.GT], F16)
            val_t = cpool.tile([P, sch.GT], F16)
            iota_t = cpool.tile([P, tw], F16)
            nc.sync.dma_start(b2_t[:], b2_d[:])
            nc.sync.dma_start(dst_t[:], dst_d[:])
            nc.sync.dma_start(val_t[:], val_d[:])
            nc.sync.dma_start(iota_t[:], iota_d[:])

            for ch in sch.chunks:
                tiles, goff, Gc = ch["tiles"], ch["goff"], ch["Gc"]
                n_t = len(tiles)
                gt = gpool.tile([P, sch.Gc_max, C], F16, tag="gt")
                nc.sync.dma_start(
                    gt[:, :Gc, :].rearrange("p g n -> p (g n)"),
                    exp_d[:, goff * C:(goff + Gc) * C])
                st = spool.tile([P, sch.Gc_max, tw], F16, tag="st")
                nc.sync.dma_start(
                    st[:, :Gc, :].rearrange("p g n -> p (g n)"),
                    sv_d[:, goff * tw:(goff + Gc) * tw])

                ps = psC.tile([tw, ct * C], F32, tag="lg")
                for tl in range(n_t):
                    lo, hi = ch["ops"][tl]
                    if hi == lo:
                        nc.vector.memset(ps[:, tl * C:(tl + 1) * C], 0.0)
                    for k in range(lo, hi):
                        nc.tensor.matmul(ps[:, tl * C:(tl + 1) * C],
                                         st[:, k, :], gt[:, k, :],
                                         start=(k == lo), stop=(k == hi - 1))
                hsb = epool.tile([tw, ct, C], F32, tag="hsb")
                flat = hsb[:].rearrange("p t n -> p (t n)")
                nc.vector.tensor_tensor(flat[:, :n_t * C], ps[:, :n_t * C],
                                        b2_t[:, :n_t * C], op=ADD)
                nm = epool.tile([tw, ct], F32, tag="nm")
                nc.vector.reduce_max(nm[:, :n_t], hsb[:, :n_t, :], axis=AX,
                                     negate=True)
                nc.vector.tensor_tensor(
                    hsb[:, :n_t, :], hsb[:, :n_t, :],
                    nm[:, :n_t].unsqueeze(2).broadcast_to([tw, n_t, C]),
                    op=ADD)
                nc.scalar.activation(flat[:, :n_t * C], flat[:, :n_t * C], EXP)
                se = epool.tile([tw, ct], F32, tag="se")
                nc.vector.reduce_sum(se[:, :n_t], hsb[:, :n_t, :], axis=AX)
                ri = epool.tile([tw, ct], F32, tag="ri")
                nc.vector.reciprocal(ri[:, :n_t], se[:, :n_t])
                nc.vector.tensor_tensor(
                    hsb[:, :n_t, :], hsb[:, :n_t, :],
                    ri[:, :n_t].unsqueeze(2).broadcast_to([tw, n_t, C]),
                    op=MUL)
                t0 = tiles[0]
                nc.scalar.dma_start(out_d[:, t0 * C:(t0 + n_t) * C],
                                    flat[:, :n_t * C])
    nc.compile()
    return nc


# ---------------------------------------------------------------- driver
LAST_PROFILE = {}


def _run(nc, in_maps, label):
    trace = os.environ.get("GCN_PROFILE") == "1"
    t0 = time.time()
    for attempt in range(3):
        try:
            res = bass_utils.run_bass_kernel_spmd(
                nc, in_maps, core_ids=list(range(len(in_maps))), trace=trace)
            break
        except Exception:
            if attempt == 2:
                raise
            time.sleep(5)
    LAST_PROFILE[label] = dict(
        wall_s=time.time() - t0,
        exec_time_ns=res.exec_time_ns,
        trace=(res.instructions_and_trace or (None, None))[1])
    return res.results


def gcn_forward(cfg: Cfg, x, edge_src, edge_dst, edge_val, W1, b1, W2, b2):
    ncr, H, C, tw, ct = cfg.n_cores, cfg.hidden, cfg.n_class, cfg.tw, cfg.ct
    x = np.asarray(x, np.float32)
    W1 = np.asarray(W1, np.float32)
    b1 = np.asarray(b1, np.float32)
    W2 = np.asarray(W2, np.float32)
    b2 = np.asarray(b2, np.float32)
    edge_src = np.asarray(edge_src, np.int64)
    edge_dst = np.asarray(edge_dst, np.int64)
    edge_val = np.asarray(edge_val, np.float32)

    t0 = time.time()
    sch = Sched(cfg, edge_src, edge_dst, edge_val)
    b1c = b1.reshape(H, 1)
    b2r = np.tile(b2, (tw, ct))
    sval = sch.sval.reshape(ncr, P, sch.GT * tw)
    iota = np.tile(np.arange(tw, dtype=np.float16), (P, 1))
    LAST_PROFILE["prep_s"] = time.time() - t0
    LAST_PROFILE["sched"] = dict(GT=sch.GT, Gc_max=sch.Gc_max,
                                 slots=sch.GT * P,
                                 n_edges=len(edge_src) // ncr)

    # K1: support = x @ W1 (own shard)
    in1 = []
    for c in range(ncr):
        xs = x[c * cfg.npc:(c + 1) * cfg.npc]
        xp = np.zeros((cfg.rows1, cfg.f_in), np.float32)
        xp[:cfg.npc] = xs
        xt = np.ascontiguousarray(
            xp.reshape(cfg.tp1, P, cfg.kb, P).transpose(3, 0, 2, 1)
              .reshape(P, cfg.tp1, cfg.f_in))
        in1.append(dict(xt=xt, w1=W1.astype(np.float16)))
    nc1 = build_k1(cfg)
    r1 = _run(nc1, in1, "k1")

    # host all-to-all #1: route support rows into slot order (fp16, no math)
    sup = np.concatenate(
        [r1[c]["sup"].reshape(P, cfg.tp1, H).transpose(1, 0, 2)
         .reshape(cfg.rows1, H)[:cfg.npc] for c in range(ncr)], axis=0)
    in2 = [dict(exp1=np.ascontiguousarray(
                    sup[sch.srcslot[c]].reshape(P, sch.GT * H)),
                sval=sval[c], dstw=sch.dst_w[c], valw=sch.val_w[c],
                iota=iota, b1c=b1c, w2c=W2)
           for c in range(ncr)]
    nc2 = build_k2(cfg, sch)
    r2 = _run(nc2, in2, "k2")

    # host all-to-all #2: route t2 rows into slot order
    t2 = np.concatenate(
        [r2[c]["t2T"].T[sch.outrow[c]] for c in range(ncr)],
        axis=0).astype(np.float16)
    in3 = [dict(exp2=np.ascontiguousarray(
                    t2[sch.srcslot[c]].reshape(P, sch.GT * C)),
                sval=sval[c], dstw=sch.dst_w[c], valw=sch.val_w[c],
                iota=iota, b2r=b2r)
           for c in range(ncr)]
    nc3 = build_k3(cfg, sch)
    r3 = _run(nc3, in3, "k3")

    out = np.concatenate(
        [r3[c]["oout"].reshape(tw, cfg.tpc, C).transpose(1, 0, 2)
         .reshape(cfg.rows_pad, C)[sch.outrow[c]] for c in range(ncr)],
        axis=0)
    return out


def kernel(x, edge_src, edge_dst, edge_val, W1, b1, W2, b2):
    cfg = Cfg()
    return gcn_forward(cfg, x, edge_src, edge_dst, edge_val, W1, b1, W2, b2)


# ---------------------------------------------------------------- self test
def _numpy_ref(x, es, ed, ev, W1, b1, W2, b2, n):
    def spmm(d):
        g = d[es] * ev[:, None]
        out = np.zeros((n, d.shape[1]), np.float32)
        np.add.at(out, ed, g)
        return out
    h = spmm(x @ W1) + b1
    h = np.maximum(h, 0)
    lg = spmm(h) @ W2 + b2
    e = np.exp(lg - lg.max(1, keepdims=True))
    return e / e.sum(1, keepdims=True)


def _selftest():
    cfg = Cfg(n_nodes=4096, f_in=256, hidden=64, n_class=16,
              n_cores=8, tw=64, ct=4, st1=4)
    rng = np.random.default_rng(1)
    n_edges = 65536
    x = rng.standard_normal((cfg.n_nodes, cfg.f_in), dtype=np.float32)
    es = rng.integers(0, cfg.n_nodes, n_edges)
    ed = rng.integers(0, cfg.n_nodes, n_edges)
    ev = rng.random(n_edges, dtype=np.float32)
    W1 = rng.standard_normal((cfg.f_in, cfg.hidden), dtype=np.float32) * 0.125
    b1 = rng.standard_normal(cfg.hidden, dtype=np.float32) * 0.01
    W2 = rng.standard_normal((cfg.hidden, cfg.n_class), dtype=np.float32) * 0.25
    b2 = rng.standard_normal(cfg.n_class, dtype=np.float32) * 0.01
    act = gcn_forward(cfg, x, es, ed, ev, W1, b1, W2, b2)
    ref = _numpy_ref(x, es, ed, ev, W1, b1, W2, b2, cfg.n_nodes)
    err = np.abs(act - ref).max()
    rel = err / np.abs(ref).max()
    print(f"selftest absmax={err:.3e} relmax={rel:.3e}")
    print("profile:", LAST_PROFILE)
    assert rel < 1.2e-2, "SELFTEST FAIL"
    print("SELFTEST PASS")


if __name__ == "__main__":
    _selftest()
